# revision 26
# baseline (speedup 1.0000x reference)
"""Causal self-attention (B=2, L=2048, HID=2048, H=16, D=128) on 8 trn2 cores.

Sharding: core c -> (batch b = c//4, head-group g = c%4 of 4 heads).
Each core computes q/k/v projections for its 512 features from its batch,
RoPE, causal attention for its 4 heads, and a partial output projection
against its Wo column slice. Host sums the 4 partials per batch.

All matmuls run in fp16 with fp32 PSUM accumulation. Softmax skips
max-subtraction (scores are O(1); exp gets a -4 bias that cancels in the
normalization). Structure per 512-column block ic: one x pass feeds V, Q, K
projections (2-bank PSUM accumulator pairs), RoPE on Q/K, then attention for
query block I=ic and the Wo partial for its rows. The softmax denominator is
computed off the PE: DVE pair-sums of the exp tiles, then a GpSimd
partition_all_reduce gives the broadcast row-sum; DVE reciprocal+multiply
normalizes. Scores for two key-tiles share one [128,1024] 2-bank PSUM tile so
a single Exp activation covers both.
"""
import numpy as np

import concourse.mybir as mybir
import concourse.tile as tile
from concourse import bacc, bass_isa
from concourse.bass_utils import run_bass_kernel_spmd

B, L, HID, H = 2, 2048, 2048, 16
D = 128               # head dim
NCORES = 8
GH = 4                # heads per core
E = GH * D            # 512 per-core qkv features
NT = HID // 128       # 16 contraction tiles
NI = L // 512         # 4 i-chunks of 512
SCALE = 1.0 / float(np.sqrt(D))

F32 = mybir.dt.float32
MULT = mybir.AluOpType.mult
ADD = mybir.AluOpType.add
IS_GE = mybir.AluOpType.is_ge
DT = mybir.dt.float16       # on-chip matmul dtype
NP_DT = np.float16
EXP_BIAS = -4.0             # exp(s*scale - 4): fp16 overflow headroom, cancels in softmax
N_WARM = 66                 # HAM warmup matmuls during initial DMA fill


def _emit(nc, tc, ctx, io):
    xT, wqT, wkT, wvT, woT, cosT, sinT, rotT, out = (
        io["xT"], io["wqT"], io["wkT"], io["wvT"], io["woT"],
        io["cosT"], io["sinT"], io["rotT"], io["out"],
    )
    xTr = xT.rearrange("(t p) i -> p t i", p=128)       # [128, 16, 2048]
    wqTr = wqT.rearrange("(t p) e -> p t e", p=128)     # [128, 16, 512]
    wkTr = wkT.rearrange("(t p) e -> p t e", p=128)
    wvTr = wvT.rearrange("(t p) e -> p t e", p=128)
    woTr = woT.rearrange("(s p) f -> p s f", p=128)     # [128, 4, 2048]

    pool = ctx.enter_context(tc.tile_pool(name="main", bufs=1))
    xpool = ctx.enter_context(tc.tile_pool(name="xsl", bufs=8))
    qpool = ctx.enter_context(tc.tile_pool(name="qp", bufs=2))
    work = ctx.enter_context(tc.tile_pool(name="work", bufs=2))
    epool = ctx.enter_context(tc.tile_pool(name="ep", bufs=1))
    obpool = ctx.enter_context(tc.tile_pool(name="ob", bufs=2))
    # PSUM: pv(2x512) + acc(2x512) + sc(2x1024) = 8 banks exactly
    ps = ctx.enter_context(tc.tile_pool(name="ps", bufs=2, space="PSUM"))

    # ---- persistent SBUF ----
    wv_sb = pool.tile([128, NT, 512], DT, tag="wv")
    wq_sb = pool.tile([128, NT, 512], DT, tag="wq")
    wk_sb = pool.tile([128, NT, 512], DT, tag="wk")
    wo_sb = pool.tile([128, GH, L], DT, tag="wo")
    cos_sb = pool.tile([128, L], DT, tag="cos")
    sin_sb = pool.tile([128, L], DT, tag="sin")
    rot = pool.tile([128, 128], DT, tag="rot")
    ebias = pool.tile([128, 1], F32, tag="ebias")
    wu = pool.tile([128, 64], DT, tag="wu")
    ones = pool.tile([128, 128], DT, tag="ones")
    v_sb = [pool.tile([128, E], DT, tag=f"v{jt}", name=f"v{jt}") for jt in range(NT)]
    kr = [pool.tile([128, L], DT, tag=f"kr{h}", name=f"kr{h}") for h in range(GH)]

    nc.gpsimd.memset(ebias[:], EXP_BIAS)
    nc.gpsimd.memset(wu[:], 0.125)
    nc.gpsimd.memset(ones[:], 1.0)

    # HAM warmup: keep the PE busy while the first x/w DMAs land. Uses the sc
    # PSUM tag (idle until attention starts).
    warm = ps.tile([128, 1024], F32, tag="sc", name="warm")
    for _ in range(N_WARM):
        nc.tensor.matmul(
            warm[0:64, 0:64], wu[:, 0:64], wu[:, 0:64],
            start=True, stop=True, skip_group_check=True,
        )

    def load_quad(ic, g):
        """mt tiles 4g..4g+3 of xT[:, ic*512:+512] in one DMA."""
        xq = xpool.tile([128, 4, 512], DT, tag="xsl", name="xq")
        nc.sync.dma_start(xq[:], xTr[:, 4 * g : 4 * g + 4, ic * 512 : (ic + 1) * 512])
        return xq

    def copy_any(k, dst, src):
        if k % 2 == 0:
            nc.scalar.copy(dst, src)
        else:
            nc.vector.tensor_copy(dst, src)

    next_quads = None
    prev_d = None
    for ic in range(NI):
        isl = slice(ic * 512, (ic + 1) * 512)
        I = ic
        nj = (I + 1) * 4

        # -- x quads for this block: prefetched at C(ic-1) start, except ic=0
        #    (DMA order interleaved with weight chunks so V matmuls start early)
        if next_quads is not None:
            quads = next_quads
        else:
            # weight loads issue from the Activation HWDGE queue so they
            # parallelize with the x-quad issues on the SP queue
            quads = []
            for g in range(4):
                if g == 0:
                    # first slices small so the first matmul starts asap
                    nc.scalar.dma_start(wv_sb[:, 0:1, :], wvTr[:, 0:1, :])
                    xq0 = xpool.tile([128, 4, 512], DT, tag="xsl", name="xq")
                    nc.sync.dma_start(xq0[:, 0:1, :], xTr[:, 0:1, 0:512])
                    nc.sync.dma_start(xq0[:, 1:4, :], xTr[:, 1:4, 0:512])
                    quads.append(xq0)
                    nc.scalar.dma_start(wv_sb[:, 1:4, :], wvTr[:, 1:4, :])
                    nc.scalar.dma_start(rot[:], rotT)
                else:
                    quads.append(load_quad(ic, g))
                    nc.scalar.dma_start(
                        wv_sb[:, 4 * g : 4 * g + 4, :], wvTr[:, 4 * g : 4 * g + 4, :]
                    )

        # -- V projection: two jt-pair passes over the quads --
        for jp in range(2):
            vps = [
                ps.tile([128, 512], F32, tag="pv" if jp == 0 else "acc", name="vp")
                for _ in range(2)
            ]
            for mt in range(NT):
                for j in range(2):
                    jt = 2 * jp + j
                    nc.tensor.matmul(
                        vps[j][:],
                        quads[mt // 4][:, mt % 4, jt * 128 : (jt + 1) * 128],
                        wv_sb[:, mt, :],
                        start=(mt == 0),
                        stop=(mt == NT - 1),
                    )
            for j in range(2):
                copy_any(j, v_sb[4 * ic + 2 * jp + j][:], vps[j][:])
            if ic == 0 and jp == 0:
                nc.scalar.dma_start(cos_sb[:], cosT)
                nc.scalar.dma_start(sin_sb[:], sinT)
                for c in range(4):
                    nc.scalar.dma_start(wq_sb[:, 4 * c : 4 * c + 4, :], wqTr[:, 4 * c : 4 * c + 4, :])

        # prefetch next block's quads now: xpool holds exactly 4+4, and SP
        # issues these ahead of C/D's out DMAs
        if ic + 1 < NI:
            next_quads = [load_quad(ic + 1, g) for g in range(4)]

        # -- Q then K projection + RoPE, per head-pair --
        qr_blk = []
        for wi, w_sb in enumerate((wq_sb, wk_sb)):
            is_k = wi == 1
            for dp in range(2):
                pps = [
                    ps.tile([128, 512], F32, tag="pv" if dp == 0 else "acc", name="pp")
                    for _ in range(2)
                ]
                for mt in range(NT):
                    for dj in range(2):
                        dt = 2 * dp + dj
                        nc.tensor.matmul(
                            pps[dj][:],
                            w_sb[:, mt, dt * 128 : (dt + 1) * 128],
                            quads[mt // 4][:, mt % 4, :],
                            start=(mt == 0),
                            stop=(mt == NT - 1),
                        )
                for dj in range(2):
                    dt = 2 * dp + dj
                    pre = work.tile([128, 512], DT, tag="pre", bufs=4, name="pre")
                    copy_any(dt, pre[:], pps[dj][:])
                    rp = ps.tile([128, 512], F32, tag="pv", name="rp")
                    nc.tensor.matmul(rp[:], rot[:], pre[:], start=True, stop=True)
                    rpc = work.tile([128, 512], DT, tag="rpc", bufs=2, name="rpc")
                    nc.scalar.copy(rpc[:], rp[:])
                    t1 = work.tile([128, 512], DT, tag="t1", name="t1")
                    nc.vector.tensor_tensor(t1[:], pre[:], cos_sb[:, isl], MULT)
                    t2 = work.tile([128, 512], DT, tag="t2", name="t2")
                    nc.vector.tensor_tensor(t2[:], rpc[:], sin_sb[:, isl], MULT)
                    if is_k:
                        nc.vector.tensor_tensor(kr[dt][:, isl], t1[:], t2[:], ADD)
                    else:
                        qb = qpool.tile([128, 512], DT, tag=f"qr{dt}", name="qb")
                        qr_blk.append(qb)
                        nc.vector.tensor_tensor(qb[:], t1[:], t2[:], ADD)
            if ic == 0 and not is_k:
                for c in range(4):
                    nc.scalar.dma_start(wk_sb[:, 4 * c : 4 * c + 4, :], wkTr[:, 4 * c : 4 * c + 4, :])
        if ic == 0:
            for s_ in range(GH):
                nc.scalar.dma_start(wo_sb[:, s_, :], woTr[:, s_, :])

        def vc0(jt):
            # diag tile jt = I*4 + t has valid columns [128*t, 512) only
            return max(0, (jt - I * 4) * 128)

        # -- attention for query block I, all heads, with the PREVIOUS block's
        #    Wo matmuls (d_step) interleaved as PE filler: the static per-engine
        #    queues mean PE stalls on exp pacing / the DVE dn chain unless
        #    independent matmuls sit between the dependent ones --
        d_step = prev_d if prev_d is not None else (lambda n: None)
        ot_blk = []
        for h in range(GH):
            qb = qr_blk[h]
            ov = ps.tile([128, 512], F32, tag="acc", name="ov")
            eacc = None
            ptiles = []
            for pt in range(nj // 2):
                sct = ps.tile([128, 1024], F32, tag="sc", name="sct")
                for j in range(2):
                    jt = 2 * pt + j
                    c0_ = vc0(jt)
                    nc.tensor.matmul(
                        sct[:, j * 512 + c0_ : (j + 1) * 512],
                        kr[h][:, jt * 128 : (jt + 1) * 128],
                        qb[:, c0_:],
                        start=True,
                        stop=True,
                    )
                ep = epool.tile([128, 1024], DT, tag=f"ep{pt}", name="ep")
                nc.scalar.activation(
                    ep[:], sct[:], mybir.ActivationFunctionType.Exp,
                    scale=SCALE, bias=ebias[:],
                )
                for j in range(2):
                    jt = 2 * pt + j
                    c0_ = vc0(jt)
                    if jt >= I * 4:
                        # zero masked cols over the whole 512 (also clears the
                        # exp'd garbage left of c0_): keep iff c - c0_ - p >= 0
                        nc.gpsimd.affine_select(
                            out=ep[:, j * 512 : (j + 1) * 512],
                            in_=ep[:, j * 512 : (j + 1) * 512],
                            compare_op=IS_GE,
                            fill=0.0,
                            base=-c0_,
                            pattern=[[1, 512]],
                            channel_multiplier=-1,
                        )
                for j in range(2):
                    jt = 2 * pt + j
                    c0_ = vc0(jt)
                    nc.tensor.matmul(
                        ov[:, c0_:],
                        v_sb[jt][:, h * 128 : (h + 1) * 128],
                        ep[:, j * 512 + c0_ : (j + 1) * 512],
                        start=(jt == 0),
                        stop=(jt == nj - 1),
                    )
                psum_t = work.tile([128, 512], DT, tag="ptile", bufs=3, name="psum_t")
                nc.vector.tensor_tensor(psum_t[:], ep[:, 0:512], ep[:, 512:1024], ADD)
                # incremental dn chain so the tail after the last exp is short
                if pt == 0:
                    ptiles.append(psum_t)
                elif pt == 1:
                    eacc = work.tile([128, 512], DT, tag="eacc", name="eacc")
                    nc.vector.tensor_tensor(eacc[:], ptiles[0][:], psum_t[:], ADD)
                else:
                    nc.vector.tensor_tensor(eacc[:], eacc[:], psum_t[:], ADD)
                # ration the filler so it lasts through the whole block
                # (I=3: 8*1 + 24*2 + 4*2 = 64 = exactly D(prev)'s matmul count)
                d_step(1 if pt % 4 == 0 else 2)
            d_step(2)
            # denominator: one ones-matmul does partition reduction + broadcast
            dn = ps.tile([128, 512], F32, tag="acc", name="dn")
            nc.tensor.matmul(dn[:], ones[:], eacc[:], start=True, stop=True)
            rbi = work.tile([128, 512], F32, tag="rbi", name="rbi")
            nc.vector.reciprocal_approx_fast(out=rbi[:], in_=dn[:])
            otb = qpool.tile([128, 512], DT, tag=f"ot{h}", name="otb")
            if ic == NI - 1 and h == GH - 1:
                # last head of the last block gates the final Wo phase: split
                # the normalize so D's first row-tiles unblock half-early
                nc.vector.tensor_tensor(otb[:, 0:256], ov[:, 0:256], rbi[:, 0:256], MULT)
                nc.vector.tensor_tensor(otb[:, 256:512], ov[:, 256:512], rbi[:, 256:512], MULT)
            else:
                nc.vector.tensor_tensor(otb[:], ov[:], rbi[:], MULT)
            ot_blk.append(otb)
        d_step(1 << 30)

        def make_d(ic_d, otb):
            state = {"g": 0, "sub": 0, "ob": None, "op": None}

            def step(n):
                emitted = 0
                while emitted < n and state["g"] < 16:
                    it4, fc = divmod(state["g"], 4)
                    if state["sub"] == 0:
                        if fc == 0:
                            state["ob"] = obpool.tile(
                                [128, 2048], DT, tag="ob", name="ob"
                            )
                        state["op"] = ps.tile([128, 512], F32, tag="pv", name="op")
                    hh = state["sub"]
                    nc.tensor.matmul(
                        state["op"][:],
                        otb[hh][:, it4 * 128 : (it4 + 1) * 128],
                        wo_sb[:, hh, fc * 512 : (fc + 1) * 512],
                        start=(hh == 0),
                        stop=(hh == GH - 1),
                    )
                    emitted += 1
                    state["sub"] += 1
                    if state["sub"] == GH:
                        state["sub"] = 0
                        copy_any(it4 + fc, state["ob"][:, fc * 512 : (fc + 1) * 512], state["op"][:])
                        state["g"] += 1
                        r0 = (4 * ic_d + it4) * 128
                        if ic_d == NI - 1 and it4 == 3:
                            # final tile: store per-chunk so the last DMA
                            # doesn't serialize behind the last copy
                            nc.sync.dma_start(
                                out[r0 : r0 + 128, fc * 512 : (fc + 1) * 512],
                                state["ob"][:, fc * 512 : (fc + 1) * 512],
                            )
                        elif fc == 3:
                            nc.sync.dma_start(out[r0 : r0 + 128, :], state["ob"][:])

            return step

        prev_d = make_d(ic, ot_blk)
    prev_d(1 << 30)


def build():
    import contextlib

    nc = bacc.Bacc("TRN2", target_bir_lowering=False, debug=False, num_devices=NCORES)
    io = {
        "xT": nc.dram_tensor("xT", [HID, L], DT, kind="ExternalInput").ap(),
        "wqT": nc.dram_tensor("wqT", [HID, E], DT, kind="ExternalInput").ap(),
        "wkT": nc.dram_tensor("wkT", [HID, E], DT, kind="ExternalInput").ap(),
        "wvT": nc.dram_tensor("wvT", [HID, E], DT, kind="ExternalInput").ap(),
        "woT": nc.dram_tensor("woT", [E, HID], DT, kind="ExternalInput").ap(),
        "cosT": nc.dram_tensor("cosT", [D, L], DT, kind="ExternalInput").ap(),
        "sinT": nc.dram_tensor("sinT", [D, L], DT, kind="ExternalInput").ap(),
        "rotT": nc.dram_tensor("rotT", [D, D], DT, kind="ExternalInput").ap(),
        "out": nc.dram_tensor("out", [L, HID], DT, kind="ExternalOutput").ap(),
    }
    with tile.TileContext(nc) as tc:
        with contextlib.ExitStack() as ctx:
            _emit(nc, tc, ctx, io)
    nc.compile()
    return nc


_NC_CACHE = []


def _rot_matrix():
    # lhsT for the rotate_half matmul: rot(q) = P @ q, lhsT = P^T.
    rotT = np.zeros((D, D), dtype=NP_DT)
    for d in range(D // 2):
        rotT[d, d + 64] = 1.0
        rotT[d + 64, d] = -1.0
    return rotT


def make_in_maps(hidden_states, cos, sin, Wq, Wk, Wv, Wo):
    f = NP_DT
    cosT = np.ascontiguousarray(cos.T.astype(f))
    sinT = np.ascontiguousarray(sin.T.astype(f))
    rotT = _rot_matrix()
    xTs = [np.ascontiguousarray(hidden_states[b].T.astype(f)) for b in range(B)]
    in_maps = []
    for c in range(NCORES):
        b, g = divmod(c, 4)
        sl = slice(g * E, (g + 1) * E)
        in_maps.append({
            "xT": xTs[b],
            "wqT": np.ascontiguousarray(Wq[sl].T.astype(f)),
            "wkT": np.ascontiguousarray(Wk[sl].T.astype(f)),
            "wvT": np.ascontiguousarray(Wv[sl].T.astype(f)),
            "woT": np.ascontiguousarray(Wo[:, sl].T.astype(f)),
            "cosT": cosT,
            "sinT": sinT,
            "rotT": rotT,
        })
    return in_maps


def kernel(hidden_states, cos, sin, Wq, Wk, Wv, Wo):
    hidden_states, cos, sin, Wq, Wk, Wv, Wo = (
        np.asarray(a) for a in (hidden_states, cos, sin, Wq, Wk, Wv, Wo)
    )
    if not _NC_CACHE:
        _NC_CACHE.append(build())
    nc = _NC_CACHE[0]
    in_maps = make_in_maps(hidden_states, cos, sin, Wq, Wk, Wv, Wo)
    r = run_bass_kernel_spmd(nc, in_maps, list(range(NCORES)))
    out = np.empty((B, L, HID), np.float32)
    for b in range(B):
        acc = r.results[4 * b]["out"].astype(np.float32)
        for g in range(1, 4):
            acc += r.results[4 * b + g]["out"].astype(np.float32)
        out[b] = acc
    return out


# revision 30
# speedup vs baseline: 1.0142x; 1.0142x over previous
"""Causal self-attention (B=2, L=2048, HID=2048, H=16, D=128) on 8 trn2 cores.

Sharding: core c -> (batch b = c//4, head-group g = c%4 of 4 heads).
Each core computes q/k/v projections for its 512 features from its batch,
RoPE, causal attention for its 4 heads, and a partial output projection
against its Wo column slice. Host sums the 4 partials per batch.

All matmuls run in fp16 with fp32 PSUM accumulation. Softmax skips
max-subtraction (scores are O(1); exp gets a -4 bias that cancels in the
normalization). Structure per 512-column block ic: one x pass feeds V, Q, K
projections (2-bank PSUM accumulator pairs), RoPE on Q/K, then attention for
query block I=ic and the Wo partial for its rows. The softmax denominator is
computed off the PE: DVE pair-sums of the exp tiles, then a GpSimd
partition_all_reduce gives the broadcast row-sum; DVE reciprocal+multiply
normalizes. Scores for two key-tiles share one [128,1024] 2-bank PSUM tile so
a single Exp activation covers both.
"""
import numpy as np

import concourse.mybir as mybir
import concourse.tile as tile
from concourse import bacc, bass_isa
from concourse.bass_utils import run_bass_kernel_spmd

B, L, HID, H = 2, 2048, 2048, 16
D = 128               # head dim
NCORES = 8
GH = 4                # heads per core
E = GH * D            # 512 per-core qkv features
NT = HID // 128       # 16 contraction tiles
NI = L // 512         # 4 i-chunks of 512
SCALE = 1.0 / float(np.sqrt(D))

F32 = mybir.dt.float32
MULT = mybir.AluOpType.mult
ADD = mybir.AluOpType.add
IS_GE = mybir.AluOpType.is_ge
DT = mybir.dt.float16       # on-chip matmul dtype
NP_DT = np.float16
EXP_BIAS = -4.0             # exp(s*scale - 4): fp16 overflow headroom, cancels in softmax
N_WARM = 66                 # HAM warmup matmuls during initial DMA fill


def _emit(nc, tc, ctx, io):
    xT, wqT, wkT, wvT, woT, cosT, sinT, rotT, out = (
        io["xT"], io["wqT"], io["wkT"], io["wvT"], io["woT"],
        io["cosT"], io["sinT"], io["rotT"], io["out"],
    )
    xTr = xT.rearrange("(t p) i -> p t i", p=128)       # [128, 16, 2048]
    wqTr = wqT.rearrange("(t p) e -> p t e", p=128)     # [128, 16, 512]
    wkTr = wkT.rearrange("(t p) e -> p t e", p=128)
    wvTr = wvT.rearrange("(t p) e -> p t e", p=128)
    woTr = woT.rearrange("(s p) f -> p s f", p=128)     # [128, 4, 2048]

    pool = ctx.enter_context(tc.tile_pool(name="main", bufs=1))
    xpool = ctx.enter_context(tc.tile_pool(name="xsl", bufs=8))
    qpool = ctx.enter_context(tc.tile_pool(name="qp", bufs=2))
    work = ctx.enter_context(tc.tile_pool(name="work", bufs=2))
    epool = ctx.enter_context(tc.tile_pool(name="ep", bufs=1))
    obpool = ctx.enter_context(tc.tile_pool(name="ob", bufs=2))
    # PSUM: pv(2x512) + acc(2x512) + sc(2x1024) = 8 banks exactly
    ps = ctx.enter_context(tc.tile_pool(name="ps", bufs=2, space="PSUM"))

    # ---- persistent SBUF ----
    wv_sb = pool.tile([128, NT, 512], DT, tag="wv")
    wq_sb = pool.tile([128, NT, 512], DT, tag="wq")
    wk_sb = pool.tile([128, NT, 512], DT, tag="wk")
    wo_sb = pool.tile([128, GH, L], DT, tag="wo")
    cos_sb = pool.tile([128, L], DT, tag="cos")
    sin_sb = pool.tile([128, L], DT, tag="sin")
    rot = pool.tile([128, 128], DT, tag="rot")
    ebias = pool.tile([128, 1], F32, tag="ebias")
    wu = pool.tile([128, 64], DT, tag="wu")
    ones = pool.tile([128, 128], DT, tag="ones")
    v_sb = [pool.tile([128, E], DT, tag=f"v{jt}", name=f"v{jt}") for jt in range(NT)]
    kr = [pool.tile([128, L], DT, tag=f"kr{h}", name=f"kr{h}") for h in range(GH)]

    nc.gpsimd.memset(ebias[:], EXP_BIAS)
    nc.gpsimd.memset(wu[:], 0.125)
    nc.gpsimd.memset(ones[:], 1.0)

    # HAM warmup: keep the PE busy while the first x/w DMAs land. Uses the sc
    # PSUM tag (idle until attention starts).
    warm = ps.tile([128, 1024], F32, tag="sc", name="warm")
    for _ in range(N_WARM):
        nc.tensor.matmul(
            warm[0:64, 0:64], wu[:, 0:64], wu[:, 0:64],
            start=True, stop=True, skip_group_check=True,
        )

    def load_quad(ic, g):
        """mt tiles 4g..4g+3 of xT[:, ic*512:+512] in one DMA."""
        xq = xpool.tile([128, 4, 512], DT, tag="xsl", name="xq")
        nc.sync.dma_start(xq[:], xTr[:, 4 * g : 4 * g + 4, ic * 512 : (ic + 1) * 512])
        return xq

    def copy_any(k, dst, src):
        if k % 2 == 0:
            nc.scalar.copy(dst, src)
        else:
            nc.vector.tensor_copy(dst, src)

    next_quads = None
    prev_d = None
    for ic in range(NI):
        isl = slice(ic * 512, (ic + 1) * 512)
        I = ic
        nj = (I + 1) * 4

        # -- x quads for this block: prefetched at C(ic-1) start, except ic=0
        #    (DMA order interleaved with weight chunks so V matmuls start early)
        if next_quads is not None:
            quads = next_quads
        else:
            quads = []
            for g in range(4):
                if g == 0:
                    # first slices small so the first matmul starts asap
                    nc.sync.dma_start(wv_sb[:, 0:1, :], wvTr[:, 0:1, :])
                    xq0 = xpool.tile([128, 4, 512], DT, tag="xsl", name="xq")
                    nc.sync.dma_start(xq0[:, 0:1, :], xTr[:, 0:1, 0:512])
                    nc.sync.dma_start(xq0[:, 1:4, :], xTr[:, 1:4, 0:512])
                    quads.append(xq0)
                    nc.sync.dma_start(wv_sb[:, 1:4, :], wvTr[:, 1:4, :])
                    nc.sync.dma_start(rot[:], rotT)
                else:
                    quads.append(load_quad(ic, g))
                    nc.sync.dma_start(
                        wv_sb[:, 4 * g : 4 * g + 4, :], wvTr[:, 4 * g : 4 * g + 4, :]
                    )

        # -- V projection: two jt-pair passes over the quads --
        for jp in range(2):
            vps = [
                ps.tile([128, 512], F32, tag="pv" if jp == 0 else "acc", name="vp")
                for _ in range(2)
            ]
            for mt in range(NT):
                for j in range(2):
                    jt = 2 * jp + j
                    nc.tensor.matmul(
                        vps[j][:],
                        quads[mt // 4][:, mt % 4, jt * 128 : (jt + 1) * 128],
                        wv_sb[:, mt, :],
                        start=(mt == 0),
                        stop=(mt == NT - 1),
                    )
            for j in range(2):
                copy_any(j, v_sb[4 * ic + 2 * jp + j][:], vps[j][:])
            if ic == 0 and jp == 0:
                nc.sync.dma_start(cos_sb[:], cosT)
                nc.sync.dma_start(sin_sb[:], sinT)
                for c in range(4):
                    nc.sync.dma_start(wq_sb[:, 4 * c : 4 * c + 4, :], wqTr[:, 4 * c : 4 * c + 4, :])

        # prefetch next block's quads now: xpool holds exactly 4+4, and SP
        # issues these ahead of C/D's out DMAs
        if ic + 1 < NI:
            next_quads = [load_quad(ic + 1, g) for g in range(4)]

        # -- Q then K projection + RoPE, per head-pair --
        qr_blk = []
        for wi, w_sb in enumerate((wq_sb, wk_sb)):
            is_k = wi == 1
            for dp in range(2):
                pps = [
                    ps.tile([128, 512], F32, tag="pv" if dp == 0 else "acc", name="pp")
                    for _ in range(2)
                ]
                for mt in range(NT):
                    for dj in range(2):
                        dt = 2 * dp + dj
                        nc.tensor.matmul(
                            pps[dj][:],
                            w_sb[:, mt, dt * 128 : (dt + 1) * 128],
                            quads[mt // 4][:, mt % 4, :],
                            start=(mt == 0),
                            stop=(mt == NT - 1),
                        )
                for dj in range(2):
                    dt = 2 * dp + dj
                    pre = work.tile([128, 512], DT, tag="pre", bufs=4, name="pre")
                    copy_any(dt, pre[:], pps[dj][:])
                    rp = ps.tile([128, 512], F32, tag="pv", name="rp")
                    nc.tensor.matmul(rp[:], rot[:], pre[:], start=True, stop=True)
                    rpc = work.tile([128, 512], DT, tag="rpc", bufs=2, name="rpc")
                    nc.scalar.copy(rpc[:], rp[:])
                    t1 = work.tile([128, 512], DT, tag="t1", name="t1")
                    nc.vector.tensor_tensor(t1[:], pre[:], cos_sb[:, isl], MULT)
                    t2 = work.tile([128, 512], DT, tag="t2", name="t2")
                    nc.vector.tensor_tensor(t2[:], rpc[:], sin_sb[:, isl], MULT)
                    if is_k:
                        nc.vector.tensor_tensor(kr[dt][:, isl], t1[:], t2[:], ADD)
                    else:
                        qb = qpool.tile([128, 512], DT, tag=f"qr{dt}", name="qb")
                        qr_blk.append(qb)
                        nc.vector.tensor_tensor(qb[:], t1[:], t2[:], ADD)
            if ic == 0 and not is_k:
                for c in range(4):
                    nc.sync.dma_start(wk_sb[:, 4 * c : 4 * c + 4, :], wkTr[:, 4 * c : 4 * c + 4, :])
        if ic == 0:
            for s_ in range(GH):
                nc.sync.dma_start(wo_sb[:, s_, :], woTr[:, s_, :])

        def vc0(jt):
            # diag tile jt = I*4 + t has valid columns [128*t, 512) only
            return max(0, (jt - I * 4) * 128)

        # -- attention for query block I, all heads, with the PREVIOUS block's
        #    Wo matmuls (d_step) interleaved as PE filler: the static per-engine
        #    queues mean PE stalls on exp pacing / the DVE dn chain unless
        #    independent matmuls sit between the dependent ones --
        d_step = prev_d if prev_d is not None else (lambda n: None)
        ot_blk = []
        for h in range(GH):
            qb = qr_blk[h]
            ov = ps.tile([128, 512], F32, tag="acc", name="ov")
            eacc = None
            ptiles = []
            for pt in range(nj // 2):
                sct = ps.tile([128, 1024], F32, tag="sc", name="sct")
                for j in range(2):
                    jt = 2 * pt + j
                    c0_ = vc0(jt)
                    nc.tensor.matmul(
                        sct[:, j * 512 + c0_ : (j + 1) * 512],
                        kr[h][:, jt * 128 : (jt + 1) * 128],
                        qb[:, c0_:],
                        start=True,
                        stop=True,
                    )
                ep = epool.tile([128, 1024], DT, tag=f"ep{pt}", name="ep")
                nc.scalar.activation(
                    ep[:], sct[:], mybir.ActivationFunctionType.Exp,
                    scale=SCALE, bias=ebias[:],
                )
                for j in range(2):
                    jt = 2 * pt + j
                    c0_ = vc0(jt)
                    if jt >= I * 4:
                        # zero masked cols over the whole 512 (also clears the
                        # exp'd garbage left of c0_): keep iff c - c0_ - p >= 0
                        nc.gpsimd.affine_select(
                            out=ep[:, j * 512 : (j + 1) * 512],
                            in_=ep[:, j * 512 : (j + 1) * 512],
                            compare_op=IS_GE,
                            fill=0.0,
                            base=-c0_,
                            pattern=[[1, 512]],
                            channel_multiplier=-1,
                        )
                for j in range(2):
                    jt = 2 * pt + j
                    c0_ = vc0(jt)
                    nc.tensor.matmul(
                        ov[:, c0_:],
                        v_sb[jt][:, h * 128 : (h + 1) * 128],
                        ep[:, j * 512 + c0_ : (j + 1) * 512],
                        start=(jt == 0),
                        stop=(jt == nj - 1),
                    )
                psum_t = work.tile([128, 512], DT, tag="ptile", bufs=3, name="psum_t")
                nc.vector.tensor_tensor(psum_t[:], ep[:, 0:512], ep[:, 512:1024], ADD)
                # incremental dn chain so the tail after the last exp is short
                if pt == 0:
                    ptiles.append(psum_t)
                elif pt == 1:
                    eacc = work.tile([128, 512], DT, tag="eacc", name="eacc")
                    nc.vector.tensor_tensor(eacc[:], ptiles[0][:], psum_t[:], ADD)
                else:
                    nc.vector.tensor_tensor(eacc[:], eacc[:], psum_t[:], ADD)
                # ration the filler so it lasts through the whole block
                # (I=3: 8*1 + 24*2 + 4*2 = 64 = exactly D(prev)'s matmul count)
                d_step(1 if pt % 4 == 0 else 2)
            d_step(2)
            # denominator: one ones-matmul does partition reduction + broadcast
            dn = ps.tile([128, 512], F32, tag="acc", name="dn")
            nc.tensor.matmul(dn[:], ones[:], eacc[:], start=True, stop=True)
            rbi = work.tile([128, 512], F32, tag="rbi", name="rbi")
            nc.vector.reciprocal_approx_fast(out=rbi[:], in_=dn[:])
            otb = qpool.tile([128, 512], DT, tag=f"ot{h}", name="otb")
            if ic == NI - 1 and h == GH - 1:
                # last head of the last block gates the final Wo phase: split
                # the normalize so D's first row-tiles unblock half-early
                nc.vector.tensor_tensor(otb[:, 0:256], ov[:, 0:256], rbi[:, 0:256], MULT)
                nc.vector.tensor_tensor(otb[:, 256:512], ov[:, 256:512], rbi[:, 256:512], MULT)
            else:
                nc.vector.tensor_tensor(otb[:], ov[:], rbi[:], MULT)
            ot_blk.append(otb)
        d_step(1 << 30)

        def make_d(ic_d, otb):
            state = {"g": 0, "sub": 0, "ob": None, "op": None}

            def step(n):
                emitted = 0
                while emitted < n and state["g"] < 16:
                    it4, fc = divmod(state["g"], 4)
                    if state["sub"] == 0:
                        if fc == 0:
                            state["ob"] = obpool.tile(
                                [128, 2048], DT, tag="ob", name="ob"
                            )
                        state["op"] = ps.tile([128, 512], F32, tag="pv", name="op")
                    hh = state["sub"]
                    nc.tensor.matmul(
                        state["op"][:],
                        otb[hh][:, it4 * 128 : (it4 + 1) * 128],
                        wo_sb[:, hh, fc * 512 : (fc + 1) * 512],
                        start=(hh == 0),
                        stop=(hh == GH - 1),
                    )
                    emitted += 1
                    state["sub"] += 1
                    if state["sub"] == GH:
                        state["sub"] = 0
                        copy_any(it4 + fc, state["ob"][:, fc * 512 : (fc + 1) * 512], state["op"][:])
                        state["g"] += 1
                        if fc == 3:
                            r0 = (4 * ic_d + it4) * 128
                            nc.sync.dma_start(out[r0 : r0 + 128, :], state["ob"][:])

            return step

        prev_d = make_d(ic, ot_blk)
    prev_d(1 << 30)


def build():
    import contextlib

    nc = bacc.Bacc("TRN2", target_bir_lowering=False, debug=False, num_devices=NCORES)
    io = {
        "xT": nc.dram_tensor("xT", [HID, L], DT, kind="ExternalInput").ap(),
        "wqT": nc.dram_tensor("wqT", [HID, E], DT, kind="ExternalInput").ap(),
        "wkT": nc.dram_tensor("wkT", [HID, E], DT, kind="ExternalInput").ap(),
        "wvT": nc.dram_tensor("wvT", [HID, E], DT, kind="ExternalInput").ap(),
        "woT": nc.dram_tensor("woT", [E, HID], DT, kind="ExternalInput").ap(),
        "cosT": nc.dram_tensor("cosT", [D, L], DT, kind="ExternalInput").ap(),
        "sinT": nc.dram_tensor("sinT", [D, L], DT, kind="ExternalInput").ap(),
        "rotT": nc.dram_tensor("rotT", [D, D], DT, kind="ExternalInput").ap(),
        "out": nc.dram_tensor("out", [L, HID], DT, kind="ExternalOutput").ap(),
    }
    with tile.TileContext(nc) as tc:
        with contextlib.ExitStack() as ctx:
            _emit(nc, tc, ctx, io)
    nc.compile()
    return nc


_NC_CACHE = []


def _rot_matrix():
    # lhsT for the rotate_half matmul: rot(q) = P @ q, lhsT = P^T.
    rotT = np.zeros((D, D), dtype=NP_DT)
    for d in range(D // 2):
        rotT[d, d + 64] = 1.0
        rotT[d + 64, d] = -1.0
    return rotT


def make_in_maps(hidden_states, cos, sin, Wq, Wk, Wv, Wo):
    f = NP_DT
    cosT = np.ascontiguousarray(cos.T.astype(f))
    sinT = np.ascontiguousarray(sin.T.astype(f))
    rotT = _rot_matrix()
    xTs = [np.ascontiguousarray(hidden_states[b].T.astype(f)) for b in range(B)]
    in_maps = []
    for c in range(NCORES):
        b, g = divmod(c, 4)
        sl = slice(g * E, (g + 1) * E)
        in_maps.append({
            "xT": xTs[b],
            "wqT": np.ascontiguousarray(Wq[sl].T.astype(f)),
            "wkT": np.ascontiguousarray(Wk[sl].T.astype(f)),
            "wvT": np.ascontiguousarray(Wv[sl].T.astype(f)),
            "woT": np.ascontiguousarray(Wo[:, sl].T.astype(f)),
            "cosT": cosT,
            "sinT": sinT,
            "rotT": rotT,
        })
    return in_maps


def kernel(hidden_states, cos, sin, Wq, Wk, Wv, Wo):
    hidden_states, cos, sin, Wq, Wk, Wv, Wo = (
        np.asarray(a) for a in (hidden_states, cos, sin, Wq, Wk, Wv, Wo)
    )
    if not _NC_CACHE:
        _NC_CACHE.append(build())
    nc = _NC_CACHE[0]
    in_maps = make_in_maps(hidden_states, cos, sin, Wq, Wk, Wv, Wo)
    r = run_bass_kernel_spmd(nc, in_maps, list(range(NCORES)))
    out = np.empty((B, L, HID), np.float32)
    for b in range(B):
        acc = r.results[4 * b]["out"].astype(np.float32)
        for g in range(1, 4):
            acc += r.results[4 * b + g]["out"].astype(np.float32)
        out[b] = acc
    return out
